# revision 1
# baseline (speedup 1.0000x reference)
"""Trainium2 Bass kernel for nn_ConnectivityGraphGenerator.

Data-parallel over batch B=128: 16 graphs per core on 8 NeuronCores.

Math restructure (vs the reference's gather/scatter formulation):
  - The edge index is the FIXED complete upper-triangular graph on N=64
    nodes, so the PyG mean aggregation is a prefix-mean over nodes:
    agg[j] = mean_{i<j} x[i]. Computed as one matmul with a constant
    [N,N] strictly-upper-triangular matrix whose column j is 1/max(j,1).
  - The edge MLP heads decompose: ef@W = h[src]@W_top + h[dst]@W_bot,
    so we compute per-node projections A=h@W_top, B=h@W_bot (+bias into
    B) and form per-edge values as a broadcast outer sum A[:,i]+B[:,j]
    via stride-0 access patterns — no gathers.
  - Everything is kept feature-major (d on partitions, (i,j) on the free
    axis) so the d-reduction is a ones-matvec on the PE.
  - The (i,j) grid is processed in 4 upper-triangular j-blocks of 16
    columns (i < 16*(k+1)), cutting dense-pair work 4096 -> 2560 and
    giving the Tile scheduler fine-grained blocks to pipeline across
    DVE/ACT/GPSIMD/PE.
  - gumbel-softmax: exp(2g) = 1/ln(u)^2 exactly, so per-edge we emit
    ez = exp(2*sigmoid(w))/ln(u)^2 and v = sim*ez; the host divides by
    the global sum of ez over real edges (softmax over the full B*E
    vector couples all cores; the division is part of unsharding).
  - softplus has no ACT table in this build: V = Ln(Exp(P) + 1) (exact;
    |P| < 3 so exp cannot overflow). The +1e-6 variance epsilon is
    dropped: min softplus here is ~0.075, so eps shifts the result by
    <2e-5 relative, far below fp32 noise in the final output.
"""

import numpy as np

import concourse.bacc as bacc
import concourse.bass as bass
import concourse.mybir as mybir
import concourse.tile as tile
from concourse.bass_utils import run_bass_kernel_spmd
from concourse.tile_rust import add_dep_helper

F32 = mybir.dt.float32
AF = mybir.ActivationFunctionType
ALU = mybir.AluOpType

B, N, T = 128, 64, 256
IN, H, OUT = N + T, 256, 128
E = N * (N - 1) // 2  # 2016
NCORES = 8
G = B // NCORES  # 16 graphs per core

# Upper-triangular j-blocks: block k covers j in [16k, 16k+16), i in [0, 16k+16)
JW = 16
NBLK = N // JW
BLOCKS = []  # (j0, iw, off, F)
_off = 0
for _k in range(NBLK):
    _iw = JW * (_k + 1)
    BLOCKS.append((JW * _k, _iw, _off, _iw * JW))
    _off += _iw * JW
NB = _off  # 2560 blocked pair slots per graph


def _body(ctx, tc):
    nc = tc.nc
    x_d = nc.dram_tensor("x", [G, N, IN], F32, kind="ExternalInput").ap()
    u_d = nc.dram_tensor("u", [G, NB], F32, kind="ExternalInput").ap()
    wg_d = nc.dram_tensor("w_gnn", [IN, H], F32, kind="ExternalInput").ap()
    bg_d = nc.dram_tensor("b_gnn", [H, 1], F32, kind="ExternalInput").ap()
    wm_d = nc.dram_tensor("w_mean", [2 * H, OUT], F32, kind="ExternalInput").ap()
    bm_d = nc.dram_tensor("b_mean", [OUT, 1], F32, kind="ExternalInput").ap()
    wv_d = nc.dram_tensor("w_var", [2 * H, OUT], F32, kind="ExternalInput").ap()
    bv_d = nc.dram_tensor("b_var", [OUT, 1], F32, kind="ExternalInput").ap()
    ww_d = nc.dram_tensor("w_w", [2 * H, 1], F32, kind="ExternalInput").ap()
    bw_d = nc.dram_tensor("b_w", [G, 1], F32, kind="ExternalInput").ap()
    v_d = nc.dram_tensor("v", [G, NB], F32, kind="ExternalOutput").ap()
    ez_d = nc.dram_tensor("ez", [G, NB], F32, kind="ExternalOutput").ap()

    singles = ctx.enter_context(tc.tile_pool(name="singles", bufs=1))

    # --- constants ---
    # lts[i, j] = 1/max(j,1) if i < j else 0  -> x.T @ lts = prefix-mean
    lts = singles.tile([N, N], F32)
    tmp = singles.tile([N, N], F32)
    nc.gpsimd.iota(
        tmp[:],
        pattern=[[1, N]],
        base=0,
        channel_multiplier=0,
        allow_small_or_imprecise_dtypes=True,
    )
    nc.vector.tensor_scalar_max(tmp[:], tmp[:], 1.0)
    nc.vector.reciprocal(tmp[:], tmp[:])
    nc.gpsimd.affine_select(
        out=lts[:],
        in_=tmp[:],
        compare_op=ALU.is_gt,
        fill=0.0,
        base=0,
        pattern=[[1, N]],  # value = j - i ; keep where > 0
        channel_multiplier=-1,
    )
    # lhsT for the d-reduction: sum_d * (-1/(2*OUT)) => -0.5*mean_d
    negones = singles.tile([OUT, 1], F32)
    nc.vector.memset(negones[:], -1.0 / (2 * OUT))

    # --- weights ---
    wg_t = singles.tile([128, 3, H], F32)
    nc.sync.dma_start(wg_t[:, 0, :], wg_d[0:128, :])
    nc.sync.dma_start(wg_t[:, 1, :], wg_d[128:256, :])
    nc.sync.dma_start(wg_t[:64, 2, :], wg_d[256:320, :])
    wm_t = singles.tile([128, 4, OUT], F32)
    wv_t = singles.tile([128, 4, OUT], F32)
    ww_t = singles.tile([128, 4, 1], F32)
    for k in range(4):
        nc.sync.dma_start(wm_t[:, k, :], wm_d[k * 128 : (k + 1) * 128, :])
        nc.sync.dma_start(wv_t[:, k, :], wv_d[k * 128 : (k + 1) * 128, :])
        nc.sync.dma_start(ww_t[:, k, :], ww_d[k * 128 : (k + 1) * 128, :])
    bg_t = singles.tile([128, 2, 1], F32)
    nc.sync.dma_start(bg_t[:, 0, :], bg_d[0:128, :])
    nc.sync.dma_start(bg_t[:, 1, :], bg_d[128:256, :])
    bm_t = singles.tile([OUT, 1], F32)
    nc.sync.dma_start(bm_t[:], bm_d[:])
    bv_t = singles.tile([OUT, 1], F32)
    nc.sync.dma_start(bv_t[:], bv_d[:])
    bw_t = singles.tile([G, 1], F32)
    nc.sync.dma_start(bw_t[:], bw_d[:])

    # --- pools ---
    xp = ctx.enter_context(tc.tile_pool(name="xp", bufs=3))
    aggp = ctx.enter_context(tc.tile_pool(name="aggp", bufs=3))
    hp = ctx.enter_context(tc.tile_pool(name="hp", bufs=3))
    headp = ctx.enter_context(tc.tile_pool(name="headp", bufs=4))
    bigp = ctx.enter_context(tc.tile_pool(name="bigp", bufs=4))
    rowp = ctx.enter_context(tc.tile_pool(name="rowp", bufs=3))
    psm = ctx.enter_context(tc.tile_pool(name="psm", bufs=5, space="PSUM"))
    psr = ctx.enter_context(tc.tile_pool(name="psr", bufs=3, space="PSUM"))
    tailp = ctx.enter_context(tc.tile_pool(name="tailp", bufs=1))

    s_all = tailp.tile([G, NB], F32)
    wa_all = tailp.tile([G, N], F32)
    wb_all = tailp.tile([G, N], F32)

    # Graph-group phasing: all Exp work for KG graphs, then all Ln work,
    # so the ACT engine switches its (per-anchor-function) table set twice
    # per group instead of twice per block. Square/Relu/Identity/Copy live
    # in every table set and never force a load.
    KG = 4
    ORDER_ACT = globals().get('_ORDER_ACT', True)

    def front(g):
        """x load, prefix-mean agg, GNN layer, head projections for graph g."""
        xt = xp.tile([N, IN], F32, tag="xt")
        nc.sync.dma_start(xt[:], x_d[g])
        agg_s = aggp.tile([128, 3, N], F32, tag="agg")
        for c in range(3):
            kp = 128 if c < 2 else 64
            ps = psm.tile([128, N], F32, tag="ps")
            nc.tensor.matmul(
                ps[:kp],
                lhsT=xt[:, c * 128 : c * 128 + kp],
                rhs=lts[:],
                start=True,
                stop=True,
            )
            nc.vector.tensor_copy(agg_s[:kp, c, :], ps[:kp])
        hT = hp.tile([128, 2, N], F32, tag="h")
        for c in range(2):
            ph = psm.tile([128, N], F32, tag="ps")
            for k in range(3):
                kp = 128 if k < 2 else 64
                nc.tensor.matmul(
                    ph[:],
                    lhsT=wg_t[:kp, k, c * 128 : (c + 1) * 128],
                    rhs=agg_s[:kp, k, :],
                    start=(k == 0),
                    stop=(k == 2),
                )
            nc.scalar.activation(hT[:, c, :], ph[:], AF.Relu, bias=bg_t[:, c, :])
        heads = {}
        for nm, w_t, koff, bias_t in (
            ("at", wm_t, 0, None),
            ("bt", wm_t, 2, bm_t),
            ("ct", wv_t, 0, None),
            ("dt", wv_t, 2, bv_t),
        ):
            dst = headp.tile([OUT, N], F32, tag=nm)
            pp = psm.tile([OUT, N], F32, tag="ps")
            for k in (0, 1):
                nc.tensor.matmul(
                    pp[:],
                    lhsT=w_t[:, koff + k, :],
                    rhs=hT[:, k, :],
                    start=(k == 0),
                    stop=(k == 1),
                )
            if bias_t is None:
                nc.scalar.activation(dst[:], pp[:], AF.Identity)
            else:
                nc.scalar.activation(dst[:], pp[:], AF.Identity, bias=bias_t[:])
            heads[nm] = dst
        # scalar edge-weight head; compute engines cannot start at arbitrary
        # partitions, so stage at partition 0 and DMA into the per-graph row.
        w_row = rowp.tile([1, 2 * N], F32, tag="wrow")
        for koff, dst_rows, col in ((0, wa_all, 0), (2, wb_all, N)):
            pw = psm.tile([1, N], F32, tag="ps")
            for k in (0, 1):
                nc.tensor.matmul(
                    pw[:],
                    lhsT=ww_t[:, koff + k, :],
                    rhs=hT[:, k, :],
                    start=(k == 0),
                    stop=(k == 1),
                )
            nc.vector.tensor_copy(w_row[:, col : col + N], pw[:])
            nc.sync.dma_start(dst_rows[g : g + 1, :], w_row[:, col : col + N])
        return heads

    prev_last_ln = [None]  # ACT-order anchor across groups
    for g0 in range(0, G, KG):
        group = list(range(g0, min(g0 + KG, G)))
        tiles = {}
        last_exp = [None]
        # phase A: fronts, M/P broadcast adds, exp(P), M^2
        for g in group:
            heads = front(g)
            at, bt, ct, dt = heads["at"], heads["bt"], heads["ct"], heads["dt"]
            for bi, (j0, iw, off, F) in enumerate(BLOCKS):
                m_t = bigp.tile([OUT, F], F32, tag=f"m{bi}")
                p_t = bigp.tile([OUT, F], F32, tag=f"p{bi}")
                m3 = m_t[:].rearrange("p (a b) -> p a b", a=iw)
                p3 = p_t[:].rearrange("p (a b) -> p a b", a=iw)
                # P on DVE so Exp (table-ordered) is fed promptly; M on
                # GPSIMD since Square runs from any table set.
                nc.gpsimd.tensor_add(
                    m3,
                    at[:, :iw, None].broadcast_to([OUT, iw, JW]),
                    bt[:, None, j0 : j0 + JW].broadcast_to([OUT, iw, JW]),
                )
                nc.vector.tensor_add(
                    p3,
                    ct[:, :iw, None].broadcast_to([OUT, iw, JW]),
                    dt[:, None, j0 : j0 + JW].broadcast_to([OUT, iw, JW]),
                )
                ei = nc.scalar.activation(p_t[:], p_t[:], AF.Exp)
                if ORDER_ACT and prev_last_ln[0] is not None:
                    # pin ACT dispatch order so Exp/Ln phases don't
                    # interleave across groups (each flip reloads the
                    # ~2.7us activation table set)
                    add_dep_helper(ei.ins, prev_last_ln[0], sync=False,
                                   reason="act table phase order")
                last_exp[0] = ei.ins
                nc.gpsimd.tensor_mul(m_t[:], m_t[:], m_t[:])
                tiles[(g, bi)] = (m_t, p_t)
        # phase B: ln -> 1/softplus, Q = M^2*Vr, PE reduce, evacuate
        for g in group:
            s_row = rowp.tile([1, NB], F32, tag="srow")
            nev = 0
            for bi, (j0, iw, off, F) in enumerate(BLOCKS):
                m_t, p_t = tiles.pop((g, bi))
                li = nc.scalar.activation(p_t[:], p_t[:], AF.Ln, bias=1.0)
                if ORDER_ACT:
                    add_dep_helper(li.ins, last_exp[0], sync=False,
                                   reason="act table phase order")
                prev_last_ln[0] = li.ins
                nc.vector.reciprocal_approx_fast(p_t[:], p_t[:])
                nc.vector.tensor_mul(m_t[:], m_t[:], p_t[:])
                for c0 in range(0, F, 512):
                    cw = min(512, F - c0)
                    sr = psr.tile([1, 512], F32, tag="sr")
                    nc.tensor.matmul(
                        sr[:, :cw],
                        lhsT=negones[:],
                        rhs=m_t[:, c0 : c0 + cw],
                        start=True,
                        stop=True,
                    )
                    dst = s_row[:, off + c0 : off + c0 + cw]
                    if nev % 3 == 2:
                        nc.vector.tensor_copy(dst, sr[:, :cw])
                    else:
                        nc.scalar.activation(dst, sr[:, :cw], AF.Copy)
                    nev += 1
            nc.sync.dma_start(s_all[g : g + 1, :], s_row[:])

    # --- tail: all 16 graphs at once, [G, NB] blocked layout ---
    u_t = tailp.tile([G, NB], F32)
    nc.sync.dma_start(u_t[:], u_d[:])
    wden = tailp.tile([G, NB], F32)
    for j0, iw, off, F in BLOCKS:
        w3 = wden[:, off : off + F].rearrange("g (a b) -> g a b", a=iw)
        nc.vector.scalar_tensor_tensor(
            out=w3,
            in0=wa_all[:, :iw, None].broadcast_to([G, iw, JW]),
            scalar=bw_t[:],
            in1=wb_all[:, None, j0 : j0 + JW].broadcast_to([G, iw, JW]),
            op0=ALU.add,
            op1=ALU.add,
        )
    ez_t = tailp.tile([G, NB], F32)
    HB = NB // 4
    for h0 in (0, HB, 2 * HB, 3 * HB):  # column halves so ACT/DVE overlap within the tail
        sl = slice(h0, h0 + HB)
        nc.scalar.activation(wden[:, sl], wden[:, sl], AF.Sigmoid)
        nc.scalar.activation(u_t[:, sl], u_t[:, sl], AF.Ln)
        nc.vector.tensor_mul(u_t[:, sl], u_t[:, sl], u_t[:, sl])  # ln(u)^2
        nc.vector.reciprocal_approx_fast(u_t[:, sl], u_t[:, sl])
        nc.scalar.activation(wden[:, sl], wden[:, sl], AF.Exp, scale=2.0)
        nc.scalar.activation(s_all[:, sl], s_all[:, sl], AF.Exp)  # sim
        nc.vector.tensor_mul(ez_t[:, sl], wden[:, sl], u_t[:, sl])  # exp(z)
        nc.sync.dma_start(ez_d[:, sl], ez_t[:, sl])
        nc.vector.tensor_mul(wden[:, sl], ez_t[:, sl], s_all[:, sl])
        nc.sync.dma_start(v_d[:, sl], wden[:, sl])


_NC_CACHE = None


def _build_nc():
    global _NC_CACHE
    if _NC_CACHE is not None:
        return _NC_CACHE
    from contextlib import ExitStack

    nc = bacc.Bacc(
        "TRN2",
        target_bir_lowering=False,
        debug=False,
        enable_asserts=False,
        num_devices=NCORES,
    )
    with tile.TileContext(nc) as tc, ExitStack() as ctx:
        _body(ctx, tc)
    nc.compile()
    _NC_CACHE = nc
    return nc


def _edge_positions():
    """Blocked-layout position of each upper-tri edge (i,j)."""
    iu0, iu1 = np.triu_indices(N, k=1)
    offs = np.array([b[2] for b in BLOCKS])
    pos = offs[iu1 // JW] + iu0 * JW + (iu1 % JW)
    return iu0, iu1, pos


def _make_in_maps(
    x_topology, x_temporal, gumbel_u, W_gnn, b_gnn, W_mean, b_mean, W_var, b_var, W_w, b_w
):
    f = np.float32
    x_full = np.concatenate(
        [np.asarray(x_topology, f), np.asarray(x_temporal, f)], axis=-1
    )  # [B, N, IN]
    _, _, pos = _edge_positions()
    u_blk = np.full((B, NB), 0.5, f)
    u_blk[:, pos] = np.asarray(gumbel_u, f).reshape(B, E)
    shared = {
        "w_gnn": np.ascontiguousarray(W_gnn, f),
        "b_gnn": np.asarray(b_gnn, f).reshape(H, 1),
        "w_mean": np.ascontiguousarray(W_mean, f),
        "b_mean": np.asarray(b_mean, f).reshape(OUT, 1),
        "w_var": np.ascontiguousarray(W_var, f),
        "b_var": np.asarray(b_var, f).reshape(OUT, 1),
        "w_w": np.ascontiguousarray(W_w, f),
        "b_w": np.full((G, 1), np.asarray(b_w, f).reshape(-1)[0], f),
    }
    in_maps = []
    for c in range(NCORES):
        sl = slice(c * G, (c + 1) * G)
        m = dict(shared)
        m["x"] = np.ascontiguousarray(x_full[sl])
        m["u"] = np.ascontiguousarray(u_blk[sl])
        in_maps.append(m)
    return in_maps


def _run_raw(in_maps, trace=False, **kw):
    nc = _build_nc()
    return run_bass_kernel_spmd(
        nc, in_maps, core_ids=list(range(NCORES)), trace=trace, **kw
    )


def kernel(**inputs) -> np.ndarray:
    in_maps = _make_in_maps(**inputs)
    res = _run_raw(in_maps)
    iu0, iu1, pos = _edge_positions()
    v = np.concatenate([r["v"] for r in res.results], axis=0)  # [B, NB]
    ez = np.concatenate([r["ez"] for r in res.results], axis=0)
    vals_v = v[:, pos]
    gsum = ez[:, pos].sum(dtype=np.float32)
    adj = np.zeros((B, N, N), np.float32)
    adj[iu0 * 0 + np.arange(B)[:, None], iu0[None, :], iu1[None, :]] = vals_v / gsum
    return adj



# revision 2
# speedup vs baseline: 1.7845x; 1.7845x over previous
"""Trainium2 Bass kernel for nn_ConnectivityGraphGenerator.

Data-parallel over batch B=128: 16 graphs per core on 8 NeuronCores.

Math restructure (v2 — PE-selector formulation):
  - Node aggregation is a prefix-mean: agg[j] = mean_{i<j} x[i], computed
    as a matmul with a constant strictly-upper-triangular [N,N] matrix.
  - Edge-head outer sums are built ON THE PE: the per-node head
    projections are computed TRANSPOSED (nodes on partitions) and stacked
    as lhsT = [A^T (rows 0:64) ; B^T (rows 64:128)] with the per-head
    bias pre-added as bias/2 on every row via a rank-1 init matmul.
    A constant 0/1 selector matrix selm[128, pairs] (rows 0:64 pick the
    src node i, rows 64:128 pick the dst node j) then yields
    M[d, pair] = A[d,i] + B[d,j] + bias[d] in ONE bf16 matmul per
    512-pair chunk (1 cycle/row on the PE).
  - softplus: V = ln(1 + exp(P)) on ACT (exp chunks read PSUM directly,
    ln runs on the whole [128, 2560] grid in one instruction).
  - square: fused into the PSUM evacuation as one DVE tensor_tensor
    (M2 = m_ps * m_ps).
  - divide: Q = M2 / V as a single GPSIMD tensor_tensor per graph.
  - d-reduction: transposed matmuls (lhsT = Q 128-pair chunk, rhs =
    -1/(2*OUT) column) write S columns straight into a held PSUM bank in
    the final [128 pairs, 20*16] tail layout — no gather/evac DMAs.
  - w-head rides along as a 129th output column of the M stack (bias
    bw/2 folded the same way); the tail z = wa_i + wb_j + bw grid is one
    more set of selector matmuls (lhsT = selm chunk, rhs = stacked w
    columns of all 16 graphs).
  - gumbel: exp(2*sigmoid(z)) / ln(u)^2 with 1/ln(u)^2 precomputed on
    the host; the global softmax denominator (sum of ez over real edges)
    is applied host-side during unsharding, as is the final / gsum.
"""

import numpy as np
import ml_dtypes

import concourse.bacc as bacc
import concourse.bass as bass
import concourse.mybir as mybir
import concourse.tile as tile
from concourse.bass_utils import run_bass_kernel_spmd

F32 = mybir.dt.float32
BF16 = mybir.dt.bfloat16
AF = mybir.ActivationFunctionType
ALU = mybir.AluOpType

B, N, T = 128, 64, 256
IN, H, OUT = N + T, 256, 128
E = N * (N - 1) // 2  # 2016
NCORES = 8
G = B // NCORES  # 16 graphs per core

# Upper-triangular j-blocks: block k covers j in [16k, 16k+16), i in [0, 16k+16)
JW = 16
NBLK = N // JW
BLOCKS = []  # (j0, iw, off, F)
_off = 0
for _k in range(NBLK):
    _iw = JW * (_k + 1)
    BLOCKS.append((JW * _k, _iw, _off, _iw * JW))
    _off += _iw * JW
NB = _off  # 2560 blocked pair slots per graph
CH = NB // 128  # 20 reduce chunks of 128 pairs
TW = CH * G  # 320 tail columns (col = c*16 + g)

bf16 = ml_dtypes.bfloat16


def _body(ctx, tc):
    nc = tc.nc
    x_d = nc.dram_tensor("x", [G, N, IN], BF16, kind="ExternalInput").ap()
    selm_d = nc.dram_tensor("selm", [128, NB], BF16, kind="ExternalInput").ap()
    lts_d = nc.dram_tensor("lts", [N, N], BF16, kind="ExternalInput").ap()
    wg_d = nc.dram_tensor("wg", [128, 3, H], BF16, kind="ExternalInput").ap()
    bgr_d = nc.dram_tensor("bgr", [1, 2, 128], BF16, kind="ExternalInput").ap()
    wmw_d = nc.dram_tensor("wmw", [128, 4, OUT + 1], BF16, kind="ExternalInput").ap()
    wv_d = nc.dram_tensor("wv", [128, 4, OUT], BF16, kind="ExternalInput").ap()
    bmw_d = nc.dram_tensor("bmw", [1, OUT + 1], BF16, kind="ExternalInput").ap()
    bv2_d = nc.dram_tensor("bv2", [1, OUT], BF16, kind="ExternalInput").ap()
    ur_d = nc.dram_tensor("ur", [128, TW], F32, kind="ExternalInput").ap()
    v_d = nc.dram_tensor("v", [128, TW], F32, kind="ExternalOutput").ap()
    ez_d = nc.dram_tensor("ez", [128, TW], F32, kind="ExternalOutput").ap()

    singles = ctx.enter_context(tc.tile_pool(name="singles", bufs=1))

    selm_t = singles.tile([128, NB], BF16)
    nc.sync.dma_start(selm_t[:], selm_d)
    lts_t = singles.tile([N, N], BF16)
    nc.sync.dma_start(lts_t[:], lts_d)
    wg_t = singles.tile([128, 3, H], BF16)
    nc.sync.dma_start(wg_t[:], wg_d)
    bgr_t = singles.tile([1, 2, 128], BF16)
    nc.sync.dma_start(bgr_t[:], bgr_d)
    wmw_t = singles.tile([128, 4, OUT + 1], BF16)
    nc.sync.dma_start(wmw_t[:], wmw_d)
    wv_t = singles.tile([128, 4, OUT], BF16)
    nc.sync.dma_start(wv_t[:], wv_d)
    bmw_t = singles.tile([1, OUT + 1], BF16)
    nc.sync.dma_start(bmw_t[:], bmw_d)
    bv2_t = singles.tile([1, OUT], BF16)
    nc.sync.dma_start(bv2_t[:], bv2_d)
    ur_t = singles.tile([128, TW], F32)
    nc.sync.dma_start(ur_t[:], ur_d)

    negones = singles.tile([128, 1], BF16)
    nc.vector.memset(negones[:], -1.0 / (2 * OUT))  # -2^-8, exact in bf16
    ones_t = singles.tile([1, N], BF16)
    nc.vector.memset(ones_t[:], 1.0)
    wab_t = singles.tile([128, G], BF16)  # stacked w-head columns, all graphs

    # held PSUM bank: S columns accumulate over the whole graph loop
    sps_pool = ctx.enter_context(tc.tile_pool(name="sps", bufs=1, space="PSUM"))
    s_ps = sps_pool.tile([128, TW], F32)

    # SBUF pools
    xp = ctx.enter_context(tc.tile_pool(name="xp", bufs=3))
    aggp = ctx.enter_context(tc.tile_pool(name="aggp", bufs=2))
    hp = ctx.enter_context(tc.tile_pool(name="hp", bufs=2))
    skp = ctx.enter_context(tc.tile_pool(name="skp", bufs=2))
    ep = ctx.enter_context(tc.tile_pool(name="ep", bufs=2))
    m2p = ctx.enter_context(tc.tile_pool(name="m2p", bufs=2))
    qp = ctx.enter_context(tc.tile_pool(name="qp", bufs=2))

    from contextlib import ExitStack
    inner = ExitStack()
    frp = inner.enter_context(tc.tile_pool(name="frp", bufs=1, space="PSUM"))
    pp = inner.enter_context(tc.tile_pool(name="pp", bufs=2, space="PSUM"))
    mp = inner.enter_context(tc.tile_pool(name="mp", bufs=2, space="PSUM"))

    PCHUNKS = [(0, 1024), (1024, 1024), (2048, 512)]

    for g in range(G):
        xt = xp.tile([N, IN], BF16, tag="xt")
        nc.sync.dma_start(xt[:], x_d[g])

        # --- prefix-mean aggregation: aggT[f, j] = mean_{i<j} x[i, f] ---
        fr = frp.tile([128, 192], F32, tag="fr")
        for c in range(3):
            kp = 128 if c < 2 else 64
            nc.tensor.matmul(
                fr[:kp, c * 64 : (c + 1) * 64],
                lhsT=xt[:, c * 128 : c * 128 + kp],
                rhs=lts_t[:],
                start=True,
                stop=True,
            )
        aggs = aggp.tile([128, 3, N], BF16, tag="agg")
        nc.vector.tensor_copy(aggs[:].rearrange("p a b -> p (a b)"), fr[:, 0:192])

        # --- GNN layer: hT[h, (c, j)] = relu(Wg^T agg + bg) ---
        fr = frp.tile([128, 192], F32, tag="fr")
        for c in range(2):
            sl = fr[:, c * 64 : (c + 1) * 64]
            nc.tensor.matmul(
                sl, lhsT=bgr_t[:, c, :], rhs=ones_t[:], start=True, stop=False
            )
            for k in range(3):
                kp = 128 if k < 2 else 64
                nc.tensor.matmul(
                    sl,
                    lhsT=wg_t[:kp, k, c * 128 : (c + 1) * 128],
                    rhs=aggs[:kp, k, :],
                    start=False,
                    stop=(k == 2),
                )
        hT = hp.tile([128, 2 * N], BF16, tag="h")
        nc.vector.tensor_scalar_max(hT[:], fr[:, 0:128], 0.0)

        # --- stacked transposed heads ---
        # stkM rows 0:64 = A^T(+bm/2, bw/2), rows 64:128 = B^T(+bm/2, bw/2);
        # col 128 = w-head. stkP likewise for the variance heads.
        fr = frp.tile([128, 192], F32, tag="fr")
        for half in range(2):
            sl = fr[half * 64 : (half + 1) * 64, 0 : OUT + 1]
            nc.tensor.matmul(
                sl, lhsT=ones_t[:], rhs=bmw_t[:], start=True, stop=False
            )
            for k in range(2):
                nc.tensor.matmul(
                    sl,
                    lhsT=hT[:, k * 64 : (k + 1) * 64],
                    rhs=wmw_t[:, half * 2 + k, :],
                    start=False,
                    stop=(k == 1),
                )
        stkM = skp.tile([128, OUT + 1], BF16, tag="skm")
        nc.vector.tensor_copy(stkM[:], fr[:, 0 : OUT + 1])
        nc.vector.tensor_copy(wab_t[:, g : g + 1], stkM[:, OUT : OUT + 1])

        fr = frp.tile([128, 192], F32, tag="fr")
        for half in range(2):
            sl = fr[half * 64 : (half + 1) * 64, 0:OUT]
            nc.tensor.matmul(
                sl, lhsT=ones_t[:], rhs=bv2_t[:], start=True, stop=False
            )
            for k in range(2):
                nc.tensor.matmul(
                    sl,
                    lhsT=hT[:, k * 64 : (k + 1) * 64],
                    rhs=wv_t[:, half * 2 + k, :],
                    start=False,
                    stop=(k == 1),
                )
        stkP = skp.tile([128, OUT], BF16, tag="skp")
        nc.vector.tensor_copy(stkP[:], fr[:, 0:OUT])

        # --- P grid -> exp -> (softplus via ln) ---
        e_t = ep.tile([128, NB], F32, tag="E")
        for o, w in PCHUNKS:
            p_ps = pp.tile([128, 1024], F32, tag="p")
            nc.tensor.matmul(
                p_ps[:, 0:512], lhsT=stkP[:], rhs=selm_t[:, o : o + 512],
                start=True, stop=True,
            )
            if w > 512:
                nc.tensor.matmul(
                    p_ps[:, 512:1024], lhsT=stkP[:],
                    rhs=selm_t[:, o + 512 : o + 1024],
                    start=True, stop=True,
                )
            nc.scalar.activation(e_t[:, o : o + w], p_ps[:, 0:w], AF.Exp)
        nc.scalar.activation(e_t[:], e_t[:], AF.Ln, bias=1.0)  # V, in place

        # --- M grid -> fused square-evacuation ---
        m2_t = m2p.tile([128, NB], F32, tag="M2")
        for t in range(5):
            m_ps = mp.tile([128, 512], F32, tag="m")
            nc.tensor.matmul(
                m_ps[:], lhsT=stkM[:, 0:OUT],
                rhs=selm_t[:, t * 512 : (t + 1) * 512],
                start=True, stop=True,
            )
            nc.vector.tensor_tensor(
                out=m2_t[:, t * 512 : (t + 1) * 512], in0=m_ps[:], in1=m_ps[:],
                op=ALU.mult,
            )

        # --- Q = M^2 / V on GPSIMD, then transposed d-reduction into S ---
        q_t = qp.tile([128, NB], BF16, tag="Q")
        nc.gpsimd.tensor_tensor(out=q_t[:], in0=m2_t[:], in1=e_t[:], op=ALU.divide)
        for c in range(CH):
            nc.tensor.matmul(
                s_ps[:, c * G + g : c * G + g + 1],
                lhsT=q_t[:, c * 128 : (c + 1) * 128],
                rhs=negones[:],
                start=True,
                stop=True,
            )

    inner.close()  # free front/m/p PSUM banks before the tail z grid

    tailp = ctx.enter_context(tc.tile_pool(name="tailp", bufs=1))
    zps_pool = ctx.enter_context(tc.tile_pool(name="zps", bufs=1, space="PSUM"))
    z_ps = zps_pool.tile([128, TW], F32)
    for c in range(CH):
        nc.tensor.matmul(
            z_ps[:, c * G : (c + 1) * G],
            lhsT=selm_t[:, c * 128 : (c + 1) * 128],
            rhs=wab_t[:],
            start=True,
            stop=True,
        )
    w_sb = tailp.tile([128, TW], F32)
    nc.scalar.activation(w_sb[:], z_ps[:], AF.Sigmoid)
    nc.scalar.activation(w_sb[:], w_sb[:], AF.Exp, scale=2.0)
    ez_sb = tailp.tile([128, TW], F32)
    nc.vector.tensor_tensor(out=ez_sb[:], in0=w_sb[:], in1=ur_t[:], op=ALU.mult)
    nc.gpsimd.dma_start(ez_d, ez_sb[:])
    sim_sb = tailp.tile([128, TW], F32)
    nc.scalar.activation(sim_sb[:], s_ps[:], AF.Exp)
    v_sb = tailp.tile([128, TW], F32)
    nc.vector.tensor_tensor(out=v_sb[:], in0=ez_sb[:], in1=sim_sb[:], op=ALU.mult)
    nc.gpsimd.dma_start(v_d, v_sb[:])


_NC_CACHE = None


def _build_nc():
    global _NC_CACHE
    if _NC_CACHE is not None:
        return _NC_CACHE
    from contextlib import ExitStack

    nc = bacc.Bacc(
        "TRN2",
        target_bir_lowering=False,
        debug=False,
        enable_asserts=False,
        num_devices=NCORES,
        num_swdge_queues=4,
    )
    with tile.TileContext(nc) as tc, ExitStack() as ctx:
        _body(ctx, tc)
    nc.compile()
    _NC_CACHE = nc
    return nc


def _edge_positions():
    """Blocked-layout position of each upper-tri edge (i,j)."""
    iu0, iu1 = np.triu_indices(N, k=1)
    offs = np.array([b[2] for b in BLOCKS])
    pos = offs[iu1 // JW] + iu0 * JW + (iu1 % JW)
    return iu0, iu1, pos


def _block_ij():
    """(i, j) node indices for every blocked pair slot."""
    i_idx = np.zeros(NB, np.int64)
    j_idx = np.zeros(NB, np.int64)
    for j0, iw, off, F in BLOCKS:
        a = np.arange(iw)[:, None]
        b = np.arange(JW)[None, :]
        sl = slice(off, off + F)
        i_idx[sl] = np.broadcast_to(a, (iw, JW)).ravel()
        j_idx[sl] = np.broadcast_to(j0 + b, (iw, JW)).ravel()
    return i_idx, j_idx


def _make_in_maps(
    x_topology, x_temporal, gumbel_u, W_gnn, b_gnn, W_mean, b_mean, W_var, b_var, W_w, b_w
):
    f = np.float32
    x_full = np.concatenate(
        [np.asarray(x_topology, f), np.asarray(x_temporal, f)], axis=-1
    ).astype(bf16)  # [B, N, IN]

    i_idx, j_idx = _block_ij()
    selm = np.zeros((128, NB), bf16)
    selm[i_idx, np.arange(NB)] = 1
    selm[64 + j_idx, np.arange(NB)] = 1

    j = np.arange(N)
    lts = ((np.arange(N)[:, None] < j[None, :]) / np.maximum(j, 1)[None, :]).astype(bf16)

    Wg = np.asarray(W_gnn, f)
    wg = np.zeros((128, 3, H), bf16)
    wg[:, 0, :] = Wg[0:128]
    wg[:, 1, :] = Wg[128:256]
    wg[:64, 2, :] = Wg[256:320]
    bgr = np.asarray(b_gnn, f).reshape(1, 2, 128).astype(bf16)

    Wm = np.asarray(W_mean, f)
    Ww = np.asarray(W_w, f).reshape(2 * H)
    wmw = np.zeros((128, 4, OUT + 1), bf16)
    for k in range(4):
        wmw[:, k, 0:OUT] = Wm[k * 128 : (k + 1) * 128]
        wmw[:, k, OUT] = Ww[k * 128 : (k + 1) * 128]
    Wv = np.asarray(W_var, f)
    wv = np.zeros((128, 4, OUT), bf16)
    for k in range(4):
        wv[:, k, :] = Wv[k * 128 : (k + 1) * 128]

    bmw = np.zeros((1, OUT + 1), f)
    bmw[0, 0:OUT] = np.asarray(b_mean, f) / 2
    bmw[0, OUT] = np.asarray(b_w, f).reshape(-1)[0] / 2
    bmw = bmw.astype(bf16)
    bv2 = (np.asarray(b_var, f).reshape(1, OUT) / 2).astype(bf16)

    # 1/ln(u)^2 in the blocked layout, then into the [128, 320] tail layout:
    # tail[p, c*G + g] = blocked[g, c*128 + p]
    _, _, pos = _edge_positions()
    u_blk = np.full((B, NB), 0.5, f)
    u_blk[:, pos] = np.asarray(gumbel_u, f).reshape(B, E)
    lu = np.log(u_blk)
    ur_all = 1.0 / (lu * lu)  # [B, NB]

    shared = {
        "selm": selm, "lts": lts, "wg": wg, "bgr": bgr,
        "wmw": wmw, "wv": wv, "bmw": bmw, "bv2": bv2,
    }
    in_maps = []
    for core in range(NCORES):
        sl = slice(core * G, (core + 1) * G)
        m = dict(shared)
        m["x"] = np.ascontiguousarray(x_full[sl])
        ur = ur_all[sl].reshape(G, CH, 128)  # [g, c, p]
        m["ur"] = np.ascontiguousarray(ur.transpose(2, 1, 0).reshape(128, TW))
        in_maps.append(m)
    return in_maps


def _run_raw(in_maps, trace=False, **kw):
    nc = _build_nc()
    return run_bass_kernel_spmd(
        nc, in_maps, core_ids=list(range(NCORES)), trace=trace, **kw
    )


def _decode(arr):
    """[128, TW] tail layout -> [G, NB] blocked rows."""
    return arr.reshape(128, CH, G).transpose(2, 1, 0).reshape(G, NB)


def kernel(**inputs) -> np.ndarray:
    in_maps = _make_in_maps(**inputs)
    res = _run_raw(in_maps)
    iu0, iu1, pos = _edge_positions()
    v = np.concatenate([_decode(r["v"]) for r in res.results], axis=0)  # [B, NB]
    ez = np.concatenate([_decode(r["ez"]) for r in res.results], axis=0)
    vals_v = v[:, pos]
    gsum = ez[:, pos].sum(dtype=np.float32)
    adj = np.zeros((B, N, N), np.float32)
    adj[np.arange(B)[:, None], iu0[None, :], iu1[None, :]] = vals_v / gsum
    return adj


# revision 3
# speedup vs baseline: 2.0570x; 1.1527x over previous
"""Trainium2 Bass kernel for nn_ConnectivityGraphGenerator.

Data-parallel over batch B=128: 16 graphs per core on 8 NeuronCores.

Math restructure (v2 — PE-selector formulation):
  - Node aggregation is a prefix-mean: agg[j] = mean_{i<j} x[i], computed
    as a matmul with a constant strictly-upper-triangular [N,N] matrix.
  - Edge-head outer sums are built ON THE PE: the per-node head
    projections are computed TRANSPOSED (nodes on partitions) and stacked
    as lhsT = [A^T (rows 0:64) ; B^T (rows 64:128)] with the per-head
    bias pre-added as bias/2 on every row via a rank-1 init matmul.
    A constant 0/1 selector matrix selm[128, pairs] (rows 0:64 pick the
    src node i, rows 64:128 pick the dst node j) then yields
    M[d, pair] = A[d,i] + B[d,j] + bias[d] in ONE bf16 matmul per
    512-pair chunk (1 cycle/row on the PE).
  - softplus: V = ln(1 + exp(P)) on ACT (exp chunks read PSUM directly,
    ln runs on the whole [128, 2560] grid in one instruction).
  - square: fused into the PSUM evacuation as one DVE tensor_tensor
    (M2 = m_ps * m_ps).
  - divide: Q = M2 / V as a single GPSIMD tensor_tensor per graph.
  - d-reduction: transposed matmuls (lhsT = Q 128-pair chunk, rhs =
    -1/(2*OUT) column) write S columns straight into a held PSUM bank in
    the final [128 pairs, 20*16] tail layout — no gather/evac DMAs.
  - w-head rides along as a 129th output column of the M stack (bias
    bw/2 folded the same way); the tail z = wa_i + wb_j + bw grid is one
    more set of selector matmuls (lhsT = selm chunk, rhs = stacked w
    columns of all 16 graphs).
  - gumbel: exp(2*sigmoid(z)) / ln(u)^2 with 1/ln(u)^2 precomputed on
    the host; the global softmax denominator (sum of ez over real edges)
    is applied host-side during unsharding, as is the final / gsum.
"""

import numpy as np
import ml_dtypes

import concourse.bacc as bacc
import concourse.bass as bass
import concourse.mybir as mybir
import concourse.tile as tile
from concourse.bass_utils import run_bass_kernel_spmd

F32 = mybir.dt.float32
BF16 = mybir.dt.bfloat16
AF = mybir.ActivationFunctionType
ALU = mybir.AluOpType

B, N, T = 128, 64, 256
IN, H, OUT = N + T, 256, 128
E = N * (N - 1) // 2  # 2016
NCORES = 8
G = B // NCORES  # 16 graphs per core

# Upper-triangular j-blocks: block k covers j in [16k, 16k+16), i in [0, 16k+16)
JW = 16
NBLK = N // JW
BLOCKS = []  # (j0, iw, off, F)
_off = 0
for _k in range(NBLK):
    _iw = JW * (_k + 1)
    BLOCKS.append((JW * _k, _iw, _off, _iw * JW))
    _off += _iw * JW
NB = _off  # 2560 blocked pair slots per graph
CH = NB // 128  # 20 reduce chunks of 128 pairs
TW = CH * G  # 320 tail columns (col = c*16 + g)

bf16 = ml_dtypes.bfloat16


def _body(ctx, tc):
    nc = tc.nc
    x_d = nc.dram_tensor("x", [G, N, IN], BF16, kind="ExternalInput").ap()
    selm_d = nc.dram_tensor("selm", [128, NB], BF16, kind="ExternalInput").ap()
    lts_d = nc.dram_tensor("lts", [N, N], BF16, kind="ExternalInput").ap()
    wg_d = nc.dram_tensor("wg", [128, 3, H], BF16, kind="ExternalInput").ap()
    bgr_d = nc.dram_tensor("bgr", [1, 2, 128], BF16, kind="ExternalInput").ap()
    wmw_d = nc.dram_tensor("wmw", [128, 4, OUT + 1], BF16, kind="ExternalInput").ap()
    wv_d = nc.dram_tensor("wv", [128, 4, OUT], BF16, kind="ExternalInput").ap()
    bmw_d = nc.dram_tensor("bmw", [1, OUT + 1], BF16, kind="ExternalInput").ap()
    bv2_d = nc.dram_tensor("bv2", [1, OUT], BF16, kind="ExternalInput").ap()
    ur_d = nc.dram_tensor("ur", [128, TW], F32, kind="ExternalInput").ap()
    v_d = nc.dram_tensor("v", [128, TW], F32, kind="ExternalOutput").ap()
    ez_d = nc.dram_tensor("ez", [128, TW], F32, kind="ExternalOutput").ap()

    singles = ctx.enter_context(tc.tile_pool(name="singles", bufs=1))

    selm_t = singles.tile([128, NB], BF16)
    nc.sync.dma_start(selm_t[:], selm_d)
    lts_t = singles.tile([N, N], BF16)
    nc.sync.dma_start(lts_t[:], lts_d)
    wg_t = singles.tile([128, 3, H], BF16)
    nc.sync.dma_start(wg_t[:], wg_d)
    bgr_t = singles.tile([1, 2, 128], BF16)
    nc.sync.dma_start(bgr_t[:], bgr_d)
    wmw_t = singles.tile([128, 4, OUT + 1], BF16)
    nc.sync.dma_start(wmw_t[:], wmw_d)
    wv_t = singles.tile([128, 4, OUT], BF16)
    nc.sync.dma_start(wv_t[:], wv_d)
    bmw_t = singles.tile([1, OUT + 1], BF16)
    nc.sync.dma_start(bmw_t[:], bmw_d)
    bv2_t = singles.tile([1, OUT], BF16)
    nc.sync.dma_start(bv2_t[:], bv2_d)
    ur_t = singles.tile([128, TW], F32)
    nc.sync.dma_start(ur_t[:], ur_d)

    negones = singles.tile([128, 1], BF16)
    nc.vector.memset(negones[:], -1.0 / (2 * OUT))  # -2^-8, exact in bf16
    ones_t = singles.tile([1, N], BF16)
    nc.vector.memset(ones_t[:], 1.0)
    wab_t = singles.tile([128, G], BF16)  # stacked w-head columns, all graphs

    # held PSUM bank: S columns accumulate over the whole graph loop
    sps_pool = ctx.enter_context(tc.tile_pool(name="sps", bufs=1, space="PSUM"))
    s_ps = sps_pool.tile([128, TW], F32)

    # SBUF pools
    xp = ctx.enter_context(tc.tile_pool(name="xp", bufs=3))
    aggp = ctx.enter_context(tc.tile_pool(name="aggp", bufs=2))
    hp = ctx.enter_context(tc.tile_pool(name="hp", bufs=2))
    skp = ctx.enter_context(tc.tile_pool(name="skp", bufs=2))
    ep = ctx.enter_context(tc.tile_pool(name="ep", bufs=2))
    m2p = ctx.enter_context(tc.tile_pool(name="m2p", bufs=2))
    qp = ctx.enter_context(tc.tile_pool(name="qp", bufs=2))

    from contextlib import ExitStack
    inner = ExitStack()
    frp = inner.enter_context(tc.tile_pool(name="frp", bufs=1, space="PSUM"))
    pp = inner.enter_context(tc.tile_pool(name="pp", bufs=2, space="PSUM"))
    mp = inner.enter_context(tc.tile_pool(name="mp", bufs=2, space="PSUM"))

    PCHUNKS = [(0, 1024), (1024, 1024), (2048, 512)]

    for g in range(G):
        xt = xp.tile([N, IN], BF16, tag="xt")
        nc.sync.dma_start(xt[:], x_d[g])

        # --- prefix-mean aggregation: aggT[f, j] = mean_{i<j} x[i, f] ---
        fr = frp.tile([128, 192], F32, tag="fr")
        for c in range(3):
            kp = 128 if c < 2 else 64
            nc.tensor.matmul(
                fr[:kp, c * 64 : (c + 1) * 64],
                lhsT=xt[:, c * 128 : c * 128 + kp],
                rhs=lts_t[:],
                start=True,
                stop=True,
            )
        aggs = aggp.tile([128, 3, N], BF16, tag="agg")
        nc.vector.tensor_copy(aggs[:].rearrange("p a b -> p (a b)"), fr[:, 0:192])

        # --- GNN layer: hT[h, (c, j)] = relu(Wg^T agg + bg) ---
        fr = frp.tile([128, 192], F32, tag="fr")
        for c in range(2):
            sl = fr[:, c * 64 : (c + 1) * 64]
            nc.tensor.matmul(
                sl, lhsT=bgr_t[:, c, :], rhs=ones_t[:], start=True, stop=False
            )
            for k in range(3):
                kp = 128 if k < 2 else 64
                nc.tensor.matmul(
                    sl,
                    lhsT=wg_t[:kp, k, c * 128 : (c + 1) * 128],
                    rhs=aggs[:kp, k, :],
                    start=False,
                    stop=(k == 2),
                )
        hT = hp.tile([128, 2 * N], BF16, tag="h")
        nc.vector.tensor_scalar_max(hT[:], fr[:, 0:128], 0.0)

        # --- stacked transposed heads ---
        # stkM rows 0:64 = A^T(+bm/2, bw/2), rows 64:128 = B^T(+bm/2, bw/2);
        # col 128 = w-head. stkP likewise for the variance heads.
        fr = frp.tile([128, 192], F32, tag="fr")
        for half in range(2):
            sl = fr[half * 64 : (half + 1) * 64, 0 : OUT + 1]
            nc.tensor.matmul(
                sl, lhsT=ones_t[:], rhs=bmw_t[:], start=True, stop=False
            )
            for k in range(2):
                nc.tensor.matmul(
                    sl,
                    lhsT=hT[:, k * 64 : (k + 1) * 64],
                    rhs=wmw_t[:, half * 2 + k, :],
                    start=False,
                    stop=(k == 1),
                )
        stkM = skp.tile([128, OUT + 1], BF16, tag="skm")
        nc.vector.tensor_copy(stkM[:], fr[:, 0 : OUT + 1])
        nc.vector.tensor_copy(wab_t[:, g : g + 1], stkM[:, OUT : OUT + 1])

        fr = frp.tile([128, 192], F32, tag="fr")
        for half in range(2):
            sl = fr[half * 64 : (half + 1) * 64, 0:OUT]
            nc.tensor.matmul(
                sl, lhsT=ones_t[:], rhs=bv2_t[:], start=True, stop=False
            )
            for k in range(2):
                nc.tensor.matmul(
                    sl,
                    lhsT=hT[:, k * 64 : (k + 1) * 64],
                    rhs=wv_t[:, half * 2 + k, :],
                    start=False,
                    stop=(k == 1),
                )
        stkP = skp.tile([128, OUT], BF16, tag="skp")
        nc.vector.tensor_copy(stkP[:], fr[:, 0:OUT])

        # --- P grid -> exp -> (softplus via ln) ---
        e_t = ep.tile([128, NB], F32, tag="E")
        for o, w in PCHUNKS:
            p_ps = pp.tile([128, 1024], F32, tag="p")
            nc.tensor.matmul(
                p_ps[:, 0:512], lhsT=stkP[:], rhs=selm_t[:, o : o + 512],
                start=True, stop=True,
            )
            if w > 512:
                nc.tensor.matmul(
                    p_ps[:, 512:1024], lhsT=stkP[:],
                    rhs=selm_t[:, o + 512 : o + 1024],
                    start=True, stop=True,
                )
            nc.scalar.activation(e_t[:, o : o + w], p_ps[:, 0:w], AF.Exp)
        nc.scalar.activation(e_t[:], e_t[:], AF.Ln, bias=1.0)  # V, in place

        # --- M grid -> fused square-evacuation ---
        m2_t = m2p.tile([128, NB], F32, tag="M2")
        for t in range(5):
            m_ps = mp.tile([128, 512], F32, tag="m")
            nc.tensor.matmul(
                m_ps[:], lhsT=stkM[:, 0:OUT],
                rhs=selm_t[:, t * 512 : (t + 1) * 512],
                start=True, stop=True,
            )
            nc.vector.tensor_tensor(
                out=m2_t[:, t * 512 : (t + 1) * 512], in0=m_ps[:], in1=m_ps[:],
                op=ALU.mult,
            )

        # --- Q = M^2 / V on GPSIMD, then transposed d-reduction into S ---
        q_t = qp.tile([128, NB], BF16, tag="Q")
        nc.gpsimd.tensor_tensor(out=q_t[:], in0=m2_t[:], in1=e_t[:], op=ALU.divide)
        for c in range(CH):
            nc.tensor.matmul(
                s_ps[:, c * G + g : c * G + g + 1],
                lhsT=q_t[:, c * 128 : (c + 1) * 128],
                rhs=negones[:],
                start=True,
                stop=True,
            )

    inner.close()  # free front/m/p PSUM banks before the tail z grid

    tailp = ctx.enter_context(tc.tile_pool(name="tailp", bufs=1))
    zps_pool = ctx.enter_context(tc.tile_pool(name="zps", bufs=1, space="PSUM"))
    z_ps = zps_pool.tile([128, TW], F32)
    for c in range(CH):
        nc.tensor.matmul(
            z_ps[:, c * G : (c + 1) * G],
            lhsT=selm_t[:, c * 128 : (c + 1) * 128],
            rhs=wab_t[:],
            start=True,
            stop=True,
        )
    w_sb = tailp.tile([128, TW], F32)
    nc.scalar.activation(w_sb[:], z_ps[:], AF.Sigmoid)
    nc.scalar.activation(w_sb[:], w_sb[:], AF.Exp, scale=2.0)
    ez_sb = tailp.tile([128, TW], F32)
    nc.vector.tensor_tensor(out=ez_sb[:], in0=w_sb[:], in1=ur_t[:], op=ALU.mult)
    nc.gpsimd.dma_start(ez_d, ez_sb[:])
    sim_sb = tailp.tile([128, TW], F32)
    nc.scalar.activation(sim_sb[:], s_ps[:], AF.Exp)
    v_sb = tailp.tile([128, TW], F32)
    nc.vector.tensor_tensor(out=v_sb[:], in0=ez_sb[:], in1=sim_sb[:], op=ALU.mult)
    nc.gpsimd.dma_start(v_d, v_sb[:])


_NC_CACHE = None


def _merge_act_table_loads(nc):
    """Collapse redundant InstLoadActFuncSet instructions.

    The compiler picks the first table set containing each activation
    function, so an Exp/Ln/Exp/... sequence flip-flops between
    `exp_and_others` and `natural_log` (1283 ns per load). One set —
    `natural_log_exp_and_others` — covers both; retarget loads to a
    covering set and drop loads whose following run is already covered.
    The inserted loads carry no semaphore waits/updates, so removal is
    sync-safe; act_func_set_id keeps the original act_info.json indexing.
    """
    from concourse.hw_specs import get_activation_tables

    tabs = list(get_activation_tables(nc.m.arch).items())
    covers = [set(fns) for _, fns in tabs]
    prefer = [i for i, (nm, _) in enumerate(tabs) if nm == "natural_log_exp_and_others"]
    order = prefer + [i for i in range(len(tabs)) if i not in prefer]
    for b in nc.m.functions[0].blocks:
        insts = b.instructions
        loads = [i for i, ins in enumerate(insts) if isinstance(ins, mybir.InstLoadActFuncSet)]
        if not loads:
            continue
        keep = [True] * len(insts)
        cur = None
        for li, i in enumerate(loads):
            j_end = loads[li + 1] if li + 1 < len(loads) else len(insts)
            funcs = {
                ins.func
                for ins in insts[i + 1 : j_end]
                if isinstance(ins, mybir.InstActivation)
            }
            if cur is not None and funcs <= covers[cur]:
                keep[i] = False
                continue
            best = next((c for c in order if funcs <= covers[c]), None)
            if best is None:
                cur = None  # unknown combo; leave the compiler's choice
            else:
                insts[i].act_func_set_id = best
                cur = best
        b.instructions = [ins for k, ins in zip(keep, insts) if k]


def _build_nc():
    global _NC_CACHE
    if _NC_CACHE is not None:
        return _NC_CACHE
    from contextlib import ExitStack

    nc = bacc.Bacc(
        "TRN2",
        target_bir_lowering=False,
        debug=False,
        enable_asserts=False,
        num_devices=NCORES,
        num_swdge_queues=4,
    )
    with tile.TileContext(nc) as tc, ExitStack() as ctx:
        _body(ctx, tc)
    nc.compile()
    _merge_act_table_loads(nc)
    _NC_CACHE = nc
    return nc


def _edge_positions():
    """Blocked-layout position of each upper-tri edge (i,j)."""
    iu0, iu1 = np.triu_indices(N, k=1)
    offs = np.array([b[2] for b in BLOCKS])
    pos = offs[iu1 // JW] + iu0 * JW + (iu1 % JW)
    return iu0, iu1, pos


def _block_ij():
    """(i, j) node indices for every blocked pair slot."""
    i_idx = np.zeros(NB, np.int64)
    j_idx = np.zeros(NB, np.int64)
    for j0, iw, off, F in BLOCKS:
        a = np.arange(iw)[:, None]
        b = np.arange(JW)[None, :]
        sl = slice(off, off + F)
        i_idx[sl] = np.broadcast_to(a, (iw, JW)).ravel()
        j_idx[sl] = np.broadcast_to(j0 + b, (iw, JW)).ravel()
    return i_idx, j_idx


def _make_in_maps(
    x_topology, x_temporal, gumbel_u, W_gnn, b_gnn, W_mean, b_mean, W_var, b_var, W_w, b_w
):
    f = np.float32
    x_full = np.concatenate(
        [np.asarray(x_topology, f), np.asarray(x_temporal, f)], axis=-1
    ).astype(bf16)  # [B, N, IN]

    i_idx, j_idx = _block_ij()
    selm = np.zeros((128, NB), bf16)
    selm[i_idx, np.arange(NB)] = 1
    selm[64 + j_idx, np.arange(NB)] = 1

    j = np.arange(N)
    lts = ((np.arange(N)[:, None] < j[None, :]) / np.maximum(j, 1)[None, :]).astype(bf16)

    Wg = np.asarray(W_gnn, f)
    wg = np.zeros((128, 3, H), bf16)
    wg[:, 0, :] = Wg[0:128]
    wg[:, 1, :] = Wg[128:256]
    wg[:64, 2, :] = Wg[256:320]
    bgr = np.asarray(b_gnn, f).reshape(1, 2, 128).astype(bf16)

    Wm = np.asarray(W_mean, f)
    Ww = np.asarray(W_w, f).reshape(2 * H)
    wmw = np.zeros((128, 4, OUT + 1), bf16)
    for k in range(4):
        wmw[:, k, 0:OUT] = Wm[k * 128 : (k + 1) * 128]
        wmw[:, k, OUT] = Ww[k * 128 : (k + 1) * 128]
    Wv = np.asarray(W_var, f)
    wv = np.zeros((128, 4, OUT), bf16)
    for k in range(4):
        wv[:, k, :] = Wv[k * 128 : (k + 1) * 128]

    bmw = np.zeros((1, OUT + 1), f)
    bmw[0, 0:OUT] = np.asarray(b_mean, f) / 2
    bmw[0, OUT] = np.asarray(b_w, f).reshape(-1)[0] / 2
    bmw = bmw.astype(bf16)
    bv2 = (np.asarray(b_var, f).reshape(1, OUT) / 2).astype(bf16)

    # 1/ln(u)^2 in the blocked layout, then into the [128, 320] tail layout:
    # tail[p, c*G + g] = blocked[g, c*128 + p]
    _, _, pos = _edge_positions()
    u_blk = np.full((B, NB), 0.5, f)
    u_blk[:, pos] = np.asarray(gumbel_u, f).reshape(B, E)
    lu = np.log(u_blk)
    ur_all = 1.0 / (lu * lu)  # [B, NB]

    shared = {
        "selm": selm, "lts": lts, "wg": wg, "bgr": bgr,
        "wmw": wmw, "wv": wv, "bmw": bmw, "bv2": bv2,
    }
    in_maps = []
    for core in range(NCORES):
        sl = slice(core * G, (core + 1) * G)
        m = dict(shared)
        m["x"] = np.ascontiguousarray(x_full[sl])
        ur = ur_all[sl].reshape(G, CH, 128)  # [g, c, p]
        m["ur"] = np.ascontiguousarray(ur.transpose(2, 1, 0).reshape(128, TW))
        in_maps.append(m)
    return in_maps


def _run_raw(in_maps, trace=False, **kw):
    nc = _build_nc()
    return run_bass_kernel_spmd(
        nc, in_maps, core_ids=list(range(NCORES)), trace=trace, **kw
    )


def _decode(arr):
    """[128, TW] tail layout -> [G, NB] blocked rows."""
    return arr.reshape(128, CH, G).transpose(2, 1, 0).reshape(G, NB)


def kernel(**inputs) -> np.ndarray:
    in_maps = _make_in_maps(**inputs)
    res = _run_raw(in_maps)
    iu0, iu1, pos = _edge_positions()
    v = np.concatenate([_decode(r["v"]) for r in res.results], axis=0)  # [B, NB]
    ez = np.concatenate([_decode(r["ez"]) for r in res.results], axis=0)
    vals_v = v[:, pos]
    gsum = ez[:, pos].sum(dtype=np.float32)
    adj = np.zeros((B, N, N), np.float32)
    adj[np.arange(B)[:, None], iu0[None, :], iu1[None, :]] = vals_v / gsum
    return adj


# revision 6
# speedup vs baseline: 2.0925x; 1.0173x over previous
"""Trainium2 Bass kernel for nn_ConnectivityGraphGenerator.

Data-parallel over batch B=128: 16 graphs per core on 8 NeuronCores.

Math restructure (v2 — PE-selector formulation):
  - Node aggregation is a prefix-mean: agg[j] = mean_{i<j} x[i], computed
    as a matmul with a constant strictly-upper-triangular [N,N] matrix.
  - Edge-head outer sums are built ON THE PE: the per-node head
    projections are computed TRANSPOSED (nodes on partitions) and stacked
    as lhsT = [A^T (rows 0:64) ; B^T (rows 64:128)] with the per-head
    bias pre-added as bias/2 on every row via a rank-1 init matmul.
    A constant 0/1 selector matrix selm[128, pairs] (rows 0:64 pick the
    src node i, rows 64:128 pick the dst node j) then yields
    M[d, pair] = A[d,i] + B[d,j] + bias[d] in ONE bf16 matmul per
    512-pair chunk (1 cycle/row on the PE).
  - softplus: V = ln(1 + exp(P)) on ACT (exp chunks read PSUM directly,
    ln runs on the whole [128, 2560] grid in one instruction).
  - square: fused into the PSUM evacuation as one DVE tensor_tensor
    (M2 = m_ps * m_ps).
  - divide: Q = M2 / V as a single GPSIMD tensor_tensor per graph.
  - d-reduction: transposed matmuls (lhsT = Q 128-pair chunk, rhs =
    -1/(2*OUT) column) write S columns straight into a held PSUM bank in
    the final [128 pairs, 20*16] tail layout — no gather/evac DMAs.
  - w-head rides along as a 129th output column of the M stack (bias
    bw/2 folded the same way); the tail z = wa_i + wb_j + bw grid is one
    more set of selector matmuls (lhsT = selm chunk, rhs = stacked w
    columns of all 16 graphs).
  - gumbel: exp(2*sigmoid(z)) / ln(u)^2 with 1/ln(u)^2 precomputed on
    the host; the global softmax denominator (sum of ez over real edges)
    is applied host-side during unsharding, as is the final / gsum.
"""

import numpy as np
import ml_dtypes

import concourse.bacc as bacc
import concourse.bass as bass
import concourse.mybir as mybir
import concourse.tile as tile
from concourse.bass_utils import run_bass_kernel_spmd

F32 = mybir.dt.float32
BF16 = mybir.dt.bfloat16
AF = mybir.ActivationFunctionType
ALU = mybir.AluOpType

B, N, T = 128, 64, 256
IN, H, OUT = N + T, 256, 128
E = N * (N - 1) // 2  # 2016
NCORES = 8
G = B // NCORES  # 16 graphs per core

# Upper-triangular j-blocks: block k covers j in [16k, 16k+16), i in [0, 16k+16)
JW = 16
NBLK = N // JW
BLOCKS = []  # (j0, iw, off, F)
_off = 0
for _k in range(NBLK):
    _iw = JW * (_k + 1)
    BLOCKS.append((JW * _k, _iw, _off, _iw * JW))
    _off += _iw * JW
NB = _off  # 2560 blocked pair slots per graph
CH = NB // 128  # 20 reduce chunks of 128 pairs
TW = CH * G  # 320 tail columns (col = c*16 + g)

bf16 = ml_dtypes.bfloat16


def _body(ctx, tc):
    nc = tc.nc
    x_d = nc.dram_tensor("x", [G, N, IN], BF16, kind="ExternalInput").ap()
    selm_d = nc.dram_tensor("selm", [128, NB], BF16, kind="ExternalInput").ap()
    lts_d = nc.dram_tensor("lts", [N, N], BF16, kind="ExternalInput").ap()
    wg_d = nc.dram_tensor("wg", [128, 3, H], BF16, kind="ExternalInput").ap()
    bgr_d = nc.dram_tensor("bgr", [1, 2, 128], BF16, kind="ExternalInput").ap()
    wmw_d = nc.dram_tensor("wmw", [128, 4, OUT + 1], BF16, kind="ExternalInput").ap()
    wv_d = nc.dram_tensor("wv", [128, 4, OUT], BF16, kind="ExternalInput").ap()
    bmw_d = nc.dram_tensor("bmw", [1, OUT + 1], BF16, kind="ExternalInput").ap()
    bv2_d = nc.dram_tensor("bv2", [1, OUT], BF16, kind="ExternalInput").ap()
    ur_d = nc.dram_tensor("ur", [128, TW], F32, kind="ExternalInput").ap()
    v_d = nc.dram_tensor("v", [128, TW], F32, kind="ExternalOutput").ap()
    ez_d = nc.dram_tensor("ez", [128, TW], F32, kind="ExternalOutput").ap()

    singles = ctx.enter_context(tc.tile_pool(name="singles", bufs=1))

    # Small weights first on the SP queue (front of graph 0 needs them);
    # the big selector right after; 1/ln(u)^2 (tail-only) via the gpsimd
    # SWDGE queues so SP stays clear for the selector.
    lts_t = singles.tile([N, N], BF16)
    nc.sync.dma_start(lts_t[:], lts_d)
    wg_t = singles.tile([128, 3, H], BF16)
    nc.sync.dma_start(wg_t[:], wg_d)
    bgr_t = singles.tile([1, 2, 128], BF16)
    nc.sync.dma_start(bgr_t[:], bgr_d)
    wmw_t = singles.tile([128, 4, OUT + 1], BF16)
    nc.sync.dma_start(wmw_t[:], wmw_d)
    wv_t = singles.tile([128, 4, OUT], BF16)
    nc.sync.dma_start(wv_t[:], wv_d)
    bmw_t = singles.tile([1, OUT + 1], BF16)
    nc.sync.dma_start(bmw_t[:], bmw_d)
    bv2_t = singles.tile([1, OUT], BF16)
    nc.sync.dma_start(bv2_t[:], bv2_d)
    selm_t = singles.tile([128, NB], BF16)
    nc.sync.dma_start(selm_t[:], selm_d)
    ur_t = singles.tile([128, TW], F32)
    nc.gpsimd.dma_start(ur_t[:], ur_d)

    negones = singles.tile([128, 1], BF16)
    nc.vector.memset(negones[:], -1.0 / (2 * OUT))  # -2^-8, exact in bf16
    ones_t = singles.tile([1, N], BF16)
    nc.vector.memset(ones_t[:], 1.0)
    wab_t = singles.tile([128, G], BF16)  # stacked w-head columns, all graphs

    # held PSUM bank: S columns accumulate over the whole graph loop
    sps_pool = ctx.enter_context(tc.tile_pool(name="sps", bufs=1, space="PSUM"))
    s_ps = sps_pool.tile([128, TW], F32)

    # SBUF pools
    xp = ctx.enter_context(tc.tile_pool(name="xp", bufs=3))
    aggp = ctx.enter_context(tc.tile_pool(name="aggp", bufs=2))
    hp = ctx.enter_context(tc.tile_pool(name="hp", bufs=2))
    skp = ctx.enter_context(tc.tile_pool(name="skp", bufs=2))
    ep = ctx.enter_context(tc.tile_pool(name="ep", bufs=2))
    m2p = ctx.enter_context(tc.tile_pool(name="m2p", bufs=2))
    qp = ctx.enter_context(tc.tile_pool(name="qp", bufs=2))

    from contextlib import ExitStack
    inner = ExitStack()
    frp = inner.enter_context(tc.tile_pool(name="frp", bufs=1, space="PSUM"))
    pp = inner.enter_context(tc.tile_pool(name="pp", bufs=2, space="PSUM"))
    mp = inner.enter_context(tc.tile_pool(name="mp", bufs=2, space="PSUM"))

    PCHUNKS = [(0, 1024), (1024, 1024), (2048, 512)]

    def emit_reduce(g, q_t):
        """Transposed d-reduction of graph g into the held S columns."""
        for c in range(CH):
            nc.tensor.matmul(
                s_ps[:, c * G + g : c * G + g + 1],
                lhsT=q_t[:, c * 128 : (c + 1) * 128],
                rhs=negones[:],
                start=True,
                stop=True,
            )

    pending = None  # (g, q_t) whose reduce is deferred one iteration

    for g in range(G):
        xt = xp.tile([N, IN], BF16, tag="xt")
        nc.gpsimd.dma_start(xt[:], x_d[g])

        # --- prefix-mean aggregation: aggT[f, j] = mean_{i<j} x[i, f] ---
        fr = frp.tile([128, 192], F32, tag="fr")
        for c in range(3):
            kp = 128 if c < 2 else 64
            nc.tensor.matmul(
                fr[:kp, c * 64 : (c + 1) * 64],
                lhsT=xt[:, c * 128 : c * 128 + kp],
                rhs=lts_t[:],
                start=True,
                stop=True,
            )
        aggs = aggp.tile([128, 3, N], BF16, tag="agg")
        nc.vector.tensor_copy(aggs[:].rearrange("p a b -> p (a b)"), fr[:, 0:192])

        # --- GNN layer: hT[h, (c, j)] = relu(Wg^T agg + bg) ---
        fr = frp.tile([128, 192], F32, tag="fr")
        for c in range(2):
            sl = fr[:, c * 64 : (c + 1) * 64]
            nc.tensor.matmul(
                sl, lhsT=bgr_t[:, c, :], rhs=ones_t[:], start=True, stop=False
            )
            for k in range(3):
                kp = 128 if k < 2 else 64
                nc.tensor.matmul(
                    sl,
                    lhsT=wg_t[:kp, k, c * 128 : (c + 1) * 128],
                    rhs=aggs[:kp, k, :],
                    start=False,
                    stop=(k == 2),
                )
        hT = hp.tile([128, 2 * N], BF16, tag="h")
        nc.vector.tensor_scalar_max(hT[:], fr[:, 0:128], 0.0)

        # --- stacked transposed heads ---
        # stkM rows 0:64 = A^T(+bm/2, bw/2), rows 64:128 = B^T(+bm/2, bw/2);
        # col 128 = w-head. stkP likewise for the variance heads.
        fr = frp.tile([128, 192], F32, tag="fr")
        for half in range(2):
            sl = fr[half * 64 : (half + 1) * 64, 0 : OUT + 1]
            nc.tensor.matmul(
                sl, lhsT=ones_t[:], rhs=bmw_t[:], start=True, stop=False
            )
            for k in range(2):
                nc.tensor.matmul(
                    sl,
                    lhsT=hT[:, k * 64 : (k + 1) * 64],
                    rhs=wmw_t[:, half * 2 + k, :],
                    start=False,
                    stop=(k == 1),
                )
        stkM = skp.tile([128, OUT + 1], BF16, tag="skm")
        nc.vector.tensor_copy(stkM[:], fr[:, 0 : OUT + 1])
        nc.vector.tensor_copy(wab_t[:, g : g + 1], stkM[:, OUT : OUT + 1])

        fr = frp.tile([128, 192], F32, tag="fr")
        for half in range(2):
            sl = fr[half * 64 : (half + 1) * 64, 0:OUT]
            nc.tensor.matmul(
                sl, lhsT=ones_t[:], rhs=bv2_t[:], start=True, stop=False
            )
            for k in range(2):
                nc.tensor.matmul(
                    sl,
                    lhsT=hT[:, k * 64 : (k + 1) * 64],
                    rhs=wv_t[:, half * 2 + k, :],
                    start=False,
                    stop=(k == 1),
                )
        stkP = skp.tile([128, OUT], BF16, tag="skp")
        nc.vector.tensor_copy(stkP[:], fr[:, 0:OUT])

        # --- P grid -> exp -> (softplus via ln) ---
        e_t = ep.tile([128, NB], F32, tag="E")
        for o, w in PCHUNKS:
            p_ps = pp.tile([128, 1024], F32, tag="p")
            nc.tensor.matmul(
                p_ps[:, 0:512], lhsT=stkP[:], rhs=selm_t[:, o : o + 512],
                start=True, stop=True,
            )
            if w > 512:
                nc.tensor.matmul(
                    p_ps[:, 512:1024], lhsT=stkP[:],
                    rhs=selm_t[:, o + 512 : o + 1024],
                    start=True, stop=True,
                )
            nc.scalar.activation(e_t[:, o : o + w], p_ps[:, 0:w], AF.Exp)
        nc.scalar.activation(e_t[:], e_t[:], AF.Ln, bias=1.0)  # V, in place

        # --- M grid -> fused square-evacuation ---
        m2_t = m2p.tile([128, NB], F32, tag="M2")
        for t in range(5):
            m_ps = mp.tile([128, 512], F32, tag="m")
            nc.tensor.matmul(
                m_ps[:], lhsT=stkM[:, 0:OUT],
                rhs=selm_t[:, t * 512 : (t + 1) * 512],
                start=True, stop=True,
            )
            nc.vector.tensor_tensor(
                out=m2_t[:, t * 512 : (t + 1) * 512], in0=m_ps[:], in1=m_ps[:],
                op=ALU.mult,
            )

        # --- Q = M^2 / V on GPSIMD; its reduce is deferred one graph so the
        # PE queue never stalls on the divide (software pipelining) ---
        q_t = qp.tile([128, NB], BF16, tag="Q")
        nc.gpsimd.tensor_tensor(out=q_t[:], in0=m2_t[:], in1=e_t[:], op=ALU.divide)
        if pending is not None:
            emit_reduce(*pending)
        pending = (g, q_t)

    emit_reduce(*pending)

    inner.close()  # free front/m/p PSUM banks before the tail z grid

    tailp = ctx.enter_context(tc.tile_pool(name="tailp", bufs=1))
    zps_pool = ctx.enter_context(tc.tile_pool(name="zps", bufs=1, space="PSUM"))
    z_ps = zps_pool.tile([128, TW], F32)
    for c in range(CH):
        nc.tensor.matmul(
            z_ps[:, c * G : (c + 1) * G],
            lhsT=selm_t[:, c * 128 : (c + 1) * 128],
            rhs=wab_t[:],
            start=True,
            stop=True,
        )
    w_sb = tailp.tile([128, TW], F32)
    nc.scalar.activation(w_sb[:], z_ps[:], AF.Sigmoid)
    nc.scalar.activation(w_sb[:], w_sb[:], AF.Exp, scale=2.0)
    ez_sb = tailp.tile([128, TW], F32)
    nc.vector.tensor_tensor(out=ez_sb[:], in0=w_sb[:], in1=ur_t[:], op=ALU.mult)
    nc.gpsimd.dma_start(ez_d, ez_sb[:])
    sim_sb = tailp.tile([128, TW], F32)
    nc.scalar.activation(sim_sb[:], s_ps[:], AF.Exp)
    v_sb = tailp.tile([128, TW], F32)
    nc.vector.tensor_tensor(out=v_sb[:], in0=ez_sb[:], in1=sim_sb[:], op=ALU.mult)
    nc.gpsimd.dma_start(v_d, v_sb[:])


_NC_CACHE = None


def _merge_act_table_loads(nc):
    """Collapse redundant InstLoadActFuncSet instructions.

    The compiler picks the first table set containing each activation
    function, so an Exp/Ln/Exp/... sequence flip-flops between
    `exp_and_others` and `natural_log` (1283 ns per load). One set —
    `natural_log_exp_and_others` — covers both; retarget loads to a
    covering set and drop loads whose following run is already covered.
    The inserted loads carry no semaphore waits/updates, so removal is
    sync-safe; act_func_set_id keeps the original act_info.json indexing.
    """
    from concourse.hw_specs import get_activation_tables

    tabs = list(get_activation_tables(nc.m.arch).items())
    covers = [set(fns) for _, fns in tabs]
    prefer = [i for i, (nm, _) in enumerate(tabs) if nm == "natural_log_exp_and_others"]
    order = prefer + [i for i in range(len(tabs)) if i not in prefer]
    for b in nc.m.functions[0].blocks:
        insts = b.instructions
        loads = [i for i, ins in enumerate(insts) if isinstance(ins, mybir.InstLoadActFuncSet)]
        if not loads:
            continue
        keep = [True] * len(insts)
        cur = None
        for li, i in enumerate(loads):
            j_end = loads[li + 1] if li + 1 < len(loads) else len(insts)
            funcs = {
                ins.func
                for ins in insts[i + 1 : j_end]
                if isinstance(ins, mybir.InstActivation)
            }
            if cur is not None and funcs <= covers[cur]:
                keep[i] = False
                continue
            best = next((c for c in order if funcs <= covers[c]), None)
            if best is None:
                cur = None  # unknown combo; leave the compiler's choice
            else:
                insts[i].act_func_set_id = best
                cur = best
        b.instructions = [ins for k, ins in zip(keep, insts) if k]


def _build_nc():
    global _NC_CACHE
    if _NC_CACHE is not None:
        return _NC_CACHE
    from contextlib import ExitStack

    nc = bacc.Bacc(
        "TRN2",
        target_bir_lowering=False,
        debug=False,
        enable_asserts=False,
        num_devices=NCORES,
        num_swdge_queues=4,
    )
    with tile.TileContext(nc) as tc, ExitStack() as ctx:
        _body(ctx, tc)
    nc.compile()
    _merge_act_table_loads(nc)
    _NC_CACHE = nc
    return nc


def _edge_positions():
    """Blocked-layout position of each upper-tri edge (i,j)."""
    iu0, iu1 = np.triu_indices(N, k=1)
    offs = np.array([b[2] for b in BLOCKS])
    pos = offs[iu1 // JW] + iu0 * JW + (iu1 % JW)
    return iu0, iu1, pos


def _block_ij():
    """(i, j) node indices for every blocked pair slot."""
    i_idx = np.zeros(NB, np.int64)
    j_idx = np.zeros(NB, np.int64)
    for j0, iw, off, F in BLOCKS:
        a = np.arange(iw)[:, None]
        b = np.arange(JW)[None, :]
        sl = slice(off, off + F)
        i_idx[sl] = np.broadcast_to(a, (iw, JW)).ravel()
        j_idx[sl] = np.broadcast_to(j0 + b, (iw, JW)).ravel()
    return i_idx, j_idx


def _make_in_maps(
    x_topology, x_temporal, gumbel_u, W_gnn, b_gnn, W_mean, b_mean, W_var, b_var, W_w, b_w
):
    f = np.float32
    x_full = np.concatenate(
        [np.asarray(x_topology, f), np.asarray(x_temporal, f)], axis=-1
    ).astype(bf16)  # [B, N, IN]

    i_idx, j_idx = _block_ij()
    selm = np.zeros((128, NB), bf16)
    selm[i_idx, np.arange(NB)] = 1
    selm[64 + j_idx, np.arange(NB)] = 1

    j = np.arange(N)
    lts = ((np.arange(N)[:, None] < j[None, :]) / np.maximum(j, 1)[None, :]).astype(bf16)

    Wg = np.asarray(W_gnn, f)
    wg = np.zeros((128, 3, H), bf16)
    wg[:, 0, :] = Wg[0:128]
    wg[:, 1, :] = Wg[128:256]
    wg[:64, 2, :] = Wg[256:320]
    bgr = np.asarray(b_gnn, f).reshape(1, 2, 128).astype(bf16)

    Wm = np.asarray(W_mean, f)
    Ww = np.asarray(W_w, f).reshape(2 * H)
    wmw = np.zeros((128, 4, OUT + 1), bf16)
    for k in range(4):
        wmw[:, k, 0:OUT] = Wm[k * 128 : (k + 1) * 128]
        wmw[:, k, OUT] = Ww[k * 128 : (k + 1) * 128]
    Wv = np.asarray(W_var, f)
    wv = np.zeros((128, 4, OUT), bf16)
    for k in range(4):
        wv[:, k, :] = Wv[k * 128 : (k + 1) * 128]

    bmw = np.zeros((1, OUT + 1), f)
    bmw[0, 0:OUT] = np.asarray(b_mean, f) / 2
    bmw[0, OUT] = np.asarray(b_w, f).reshape(-1)[0] / 2
    bmw = bmw.astype(bf16)
    bv2 = (np.asarray(b_var, f).reshape(1, OUT) / 2).astype(bf16)

    # 1/ln(u)^2 in the blocked layout, then into the [128, 320] tail layout:
    # tail[p, c*G + g] = blocked[g, c*128 + p]
    _, _, pos = _edge_positions()
    u_blk = np.full((B, NB), 0.5, f)
    u_blk[:, pos] = np.asarray(gumbel_u, f).reshape(B, E)
    lu = np.log(u_blk)
    ur_all = 1.0 / (lu * lu)  # [B, NB]

    shared = {
        "selm": selm, "lts": lts, "wg": wg, "bgr": bgr,
        "wmw": wmw, "wv": wv, "bmw": bmw, "bv2": bv2,
    }
    in_maps = []
    for core in range(NCORES):
        sl = slice(core * G, (core + 1) * G)
        m = dict(shared)
        m["x"] = np.ascontiguousarray(x_full[sl])
        ur = ur_all[sl].reshape(G, CH, 128)  # [g, c, p]
        m["ur"] = np.ascontiguousarray(ur.transpose(2, 1, 0).reshape(128, TW))
        in_maps.append(m)
    return in_maps


def _run_raw(in_maps, trace=False, **kw):
    nc = _build_nc()
    return run_bass_kernel_spmd(
        nc, in_maps, core_ids=list(range(NCORES)), trace=trace, **kw
    )


def _decode(arr):
    """[128, TW] tail layout -> [G, NB] blocked rows."""
    return arr.reshape(128, CH, G).transpose(2, 1, 0).reshape(G, NB)


def kernel(**inputs) -> np.ndarray:
    in_maps = _make_in_maps(**inputs)
    res = _run_raw(in_maps)
    iu0, iu1, pos = _edge_positions()
    v = np.concatenate([_decode(r["v"]) for r in res.results], axis=0)  # [B, NB]
    ez = np.concatenate([_decode(r["ez"]) for r in res.results], axis=0)
    vals_v = v[:, pos]
    gsum = ez[:, pos].sum(dtype=np.float32)
    adj = np.zeros((B, N, N), np.float32)
    adj[np.arange(B)[:, None], iu0[None, :], iu1[None, :]] = vals_v / gsum
    return adj
